# revision 35
# baseline (speedup 1.0000x reference)
"""Trainium2 Bass kernel for attention pooling (sparse_attention).

Computation (per batch b):
    proj_feat = einsum("fl,af->la", features[b], W_feat)        # [L, A]
    p         = z[b] @ W_inp.T + b_inp                          # [A]
    att       = relu(proj_feat + p)                             # [L, A]
    scores    = att @ v_atten                                   # [L]
    alpha     = softmax(scores)                                 # [L]
    ctx       = features[b] @ alpha                             # [F]

Sharding: data-parallel over batch B=16 across 8 cores (2 batches/core).
All parameters replicated. Features are converted to bf16 on the host and
staged in two layouts (natural [F, L] for the projection matmul, and a
transposed [p, c, F] layout for the context matmul) so both big
contractions run on the TensorEngine with the contraction dim on
partitions.

Softmax skips max-subtraction (scores are O(+-5); exp is safe in fp32).

The context matmul is PIPELINED into phase 1: score tile t (l in
[512t, 512t+512)) scatters into scores2d[:, 4t:4t+4] (l = 512t + 128j + p),
is exp'ed per column-block, and immediately feeds 4 ctx matmuls. Emission
lags (exp 2 tiles, ctx 4 tiles behind the score tile) keep every engine's
FIFO stream free of long semaphore waits.
"""

import os
import sys

for _p in ("/root/.axon_site/_ro/trn_rl_repo", "/opt/trn_rl_repo"):
    if os.path.isdir(_p) and _p not in sys.path:
        sys.path.append(_p)

import ml_dtypes
import numpy as np

import concourse.bass as bass  # noqa: F401  (registers engine classes)
import concourse.tile as tile
from concourse import bacc, mybir
from concourse.bass_utils import run_bass_kernel_spmd
from concourse.tile import add_dep_helper

BF16 = ml_dtypes.bfloat16

N_CORES = 8
B = 16
B_LOC = B // N_CORES  # 2 batches per core
F = 512
L = 8192
A = 256
I = 512

P = 128
NKF = F // P          # 4 F-chunks (contraction for proj matmul)
NA = A // P           # 2 A-chunks
LC = 2048             # fn DMA chunk along L
TS = 512              # matmul L-subtile (one PSUM bank)
NT = L // TS          # 16 score tiles per batch
CW = NT * 4           # 64 ctx columns; c = 4t + j, l = 512t + 128j + p
QF = CW // 4          # c-blocks per ft DMA quarter

EXP_LAG = 3           # exp of tile t emitted with tile t+EXP_LAG
CTX_LAG = 6           # ctx block of tile t emitted with tile t+CTX_LAG

_CACHE = {}


def _build():
    if "nc" in _CACHE:
        return _CACHE["nc"]

    f32 = mybir.dt.float32
    bf16 = mybir.dt.bfloat16
    AF = mybir.ActivationFunctionType

    nc = bacc.Bacc("TRN2", target_bir_lowering=False, debug=False)

    fn = nc.dram_tensor("fn", [B_LOC, F, L], bf16, kind="ExternalInput")
    ft = nc.dram_tensor("ft", [B_LOC, P, CW, F], bf16, kind="ExternalInput")
    wf = nc.dram_tensor("wf", [P, NKF, A], bf16, kind="ExternalInput")
    wi = nc.dram_tensor("wi", [P, NKF, A], f32, kind="ExternalInput")
    zt = nc.dram_tensor("zt", [P, NKF, B_LOC], f32, kind="ExternalInput")
    bic = nc.dram_tensor("bic", [P, NA], f32, kind="ExternalInput")
    vv = nc.dram_tensor("v", [P, NA], bf16, kind="ExternalInput")
    octx = nc.dram_tensor("ctx", [B_LOC, F], f32, kind="ExternalOutput")
    oalpha = nc.dram_tensor("alpha", [B_LOC, L], f32, kind="ExternalOutput")

    with tile.TileContext(nc) as tc:
        with (
            tc.tile_pool(name="consts", bufs=1) as consts,
            tc.tile_pool(name="fnp", bufs=4) as fnp,
            tc.tile_pool(name="ftp", bufs=6) as ftp,
            tc.tile_pool(name="attp", bufs=4) as attp,
            tc.tile_pool(name="smallp", bufs=4) as smallp,
            tc.tile_pool(name="batchp", bufs=2) as batchp,
            tc.tile_pool(name="psS", bufs=4, space="PSUM") as psS,
            tc.tile_pool(name="psV", bufs=1, space="PSUM") as psV,
            tc.tile_pool(name="psC", bufs=2, space="PSUM") as psC,
            tc.tile_pool(name="psT", bufs=1, space="PSUM") as psT,
        ):
            # ---- constants / setup ----
            wf_sb = consts.tile([P, NKF * A], bf16)
            nc.scalar.dma_start(wf_sb[:], wf.ap()[:, :, :])
            wi_sb = consts.tile([P, NKF * A], f32)
            nc.scalar.dma_start(wi_sb[:], wi.ap()[:, :, :])
            zt_sb = consts.tile([P, NKF * B_LOC], f32)
            nc.scalar.dma_start(zt_sb[:], zt.ap()[:, :, :])
            bic_sb = consts.tile([P, NA], f32)
            nc.scalar.dma_start(bic_sb[:], bic.ap()[:, :])
            v_sb = consts.tile([P, NA], bf16)
            nc.scalar.dma_start(v_sb[:], vv.ap()[:, :])
            ones_col = consts.tile([P, 1], f32)
            nc.any.memset(ones_col[:], 1.0)
            ones_row = consts.tile([1, P], f32)
            nc.any.memset(ones_row[:1], 1.0)

            # proj_inp^T: pT[p, a*B_LOC + b] = (z @ W_inp.T + b_inp)[b, a*128+p]
            pT_sb = consts.tile([P, NA * B_LOC], f32)
            for a in range(NA):
                pt_ps = psT.tile([P, B_LOC], f32, tag="tiny")
                for ki in range(NKF):
                    nc.tensor.matmul(
                        pt_ps[:, :B_LOC],
                        wi_sb[:, ki * A + a * P : ki * A + a * P + P],
                        zt_sb[:, ki * B_LOC : (ki + 1) * B_LOC],
                        start=(ki == 0),
                        stop=(ki == NKF - 1),
                    )
                nc.scalar.activation(
                    pT_sb[:, a * B_LOC : (a + 1) * B_LOC],
                    pt_ps[:, :B_LOC],
                    AF.Identity,
                    bias=bic_sb[:, a : a + 1],
                )

            # fn chunk plans (elements along L); each chunk is ONE
            # consolidated DMA [128, 4*lsz] (free = (kf, l)) on the sync ring,
            # which carries nothing else, so ring FIFO == consumption order.
            chunk_plans = {
                0: [1024, 1024, 2048, 2048, 2048],
                1: [2048, 2048, 2048, 2048],
            }
            # ft rides the scalar HWDGE ring in 2 MB quarters with
            # modeled-time floors (ms) pacing them behind the fn ramp.
            ft_floor = {0: [0.016, 0.027, 0.038, 0.049],
                        1: [0.068, 0.079, 0.090, 0.101]}

            st = {}  # per-batch state

            def start_batch(b):
                s = {}
                s["ft_q"] = []
                for qi in range(4):
                    t = ftp.tile([P, QF * F], bf16, tag="ftq")
                    with tc.tile_wait_until(ft_floor[b][qi]):
                        nc.scalar.dma_start(
                            t[:],
                            ft.ap()[b, :, qi * QF : (qi + 1) * QF, :],
                        )
                    s["ft_q"].append(t)
                s["scores2d"] = batchp.tile(
                    [P, CW], f32, tag="scores2d", name="scores2d"
                )
                s["w_sb"] = batchp.tile([P, CW], f32, tag="w_sb", name="w_sb")
                s["rowsums"] = batchp.tile(
                    [P, NT], f32, tag="rowsums", name="rowsums"
                )
                s["w16"] = batchp.tile([P, CW], bf16, tag="w16", name="w16")
                s["fn_src"] = fn.ap()[b].rearrange("(kf p) l -> p kf l", p=P)
                s["l0"] = 0
                s["relu_insts"] = {}
                s["pc"] = None
                st[b] = s

            def emit_exp(b, t):
                # exp of score tile t -> w_sb/w16 columns 4t..4t+4.
                # Scheduler-order it after the newest relu so the ACT stream
                # never parks on the scatter semaphore.
                s = st[b]
                cs = slice(4 * t, 4 * t + 4)
                e = nc.scalar.activation(
                    s["w_sb"][:, cs], s["scores2d"][:, cs], AF.Exp,
                    accum_out=s["rowsums"][:, t : t + 1],
                )
                last_relu = s["relu_insts"].get(max(s["relu_insts"]))
                if last_relu is not None:
                    add_dep_helper(e.ins, last_relu.ins, sync=False,
                                   reason="exp ordered after newest relu")
                nc.vector.tensor_copy(s["w16"][:, cs], s["w_sb"][:, cs])

            def emit_ctx_block(b, t):
                s = st[b]
                if s["pc"] is None:
                    s["pc"] = psC.tile([1, F], f32, tag="pc", name="pc")
                pc = s["pc"]
                for j in range(4):
                    c = 4 * t + j
                    nc.tensor.matmul(
                        pc[:1],
                        s["w16"][:, c : c + 1],
                        s["ft_q"][c // QF][:, (c % QF) * F : (c % QF + 1) * F],
                        start=(c == 0),
                        stop=(c == CW - 1),
                    )

            def tile_core(b, t_idx, fn_t, lsz, ts_):
                s = st[b]
                psc = psV.tile([1, TS], f32, tag="psc", name="psc")
                for a in range(NA):
                    p1 = psS.tile([P, TS], f32, tag="p1", name="p1")
                    for kf in range(NKF):
                        nc.tensor.matmul(
                            p1[:],
                            wf_sb[:, kf * A + a * P : kf * A + a * P + P],
                            fn_t[:, kf * lsz + ts_ * TS : kf * lsz + (ts_ + 1) * TS],
                            start=(kf == 0),
                            stop=(kf == NKF - 1),
                        )
                    att_t = attp.tile([P, TS], bf16, tag="att", name="att")
                    r = nc.scalar.activation(
                        att_t[:],
                        p1[:],
                        AF.Relu,
                        bias=pT_sb[:, a * B_LOC + b : a * B_LOC + b + 1],
                    )
                    s["relu_insts"][(t_idx, a)] = r
                    nc.tensor.matmul(
                        psc[:1],
                        v_sb[:, a : a + 1],
                        att_t[:],
                        start=(a == 0),
                        stop=(a == NA - 1),
                    )
                sc_row = smallp.tile([1, TS], f32, tag="srow", name="srow")
                # drain with a (j p) -> (p j) permuted read so sc_row is
                # p-major and the scatter below is a plain partition-spread
                nc.vector.tensor_copy(
                    sc_row[:1].rearrange("o (p j) -> o p j", j=4),
                    psc[:1].rearrange("o (j p) -> o p j", j=4),
                )
                # scatter tile t -> scores2d[:, 4t:4t+4]  (l = 512t + 128j + p)
                nc.gpsimd.dma_start(
                    s["scores2d"][:, 4 * t_idx : 4 * t_idx + 4],
                    sc_row[:1],
                )
                if t_idx - EXP_LAG >= 0:
                    emit_exp(b, t_idx - EXP_LAG)
                if t_idx - CTX_LAG >= 0:
                    emit_ctx_block(b, t_idx - CTX_LAG)

            def phase1_chunk(b, lc):
                s = st[b]
                lsz = chunk_plans[b][lc]
                l0 = s["l0"]
                fn_t = fnp.tile([P, NKF * LC], bf16, tag="fn", name="fn")
                nc.sync.dma_start(
                    fn_t[:, : NKF * lsz],
                    s["fn_src"][:, :, l0 : l0 + lsz],
                )
                for ts_ in range(lsz // TS):
                    tile_core(b, l0 // TS + ts_, fn_t, lsz, ts_)
                s["l0"] = l0 + lsz

            def finish_batch(b):
                s = st[b]
                for t in range(NT - EXP_LAG, NT):
                    emit_exp(b, t)
                for t in range(NT - CTX_LAG, NT):
                    emit_ctx_block(b, t)
                # softmax tail: Z, 1/Z, alpha, ctx scale
                rowsum = batchp.tile([P, 1], f32, tag="rowsum", name="rowsum")
                nc.vector.tensor_reduce(
                    rowsum[:, 0:1], s["rowsums"][:],
                    mybir.AxisListType.X, mybir.AluOpType.add,
                )
                zp = psT.tile([P, B_LOC], f32, tag="tiny", name="zp")
                nc.tensor.matmul(
                    zp[:1, :1], ones_col[:], rowsum[:], start=True, stop=True
                )
                recip = batchp.tile([1, 1], f32, tag="recip", name="recip")
                nc.vector.reciprocal(recip[:1, :1], zp[:1, :1])
                rp = psT.tile([P, B_LOC], f32, tag="tiny", name="rp")
                nc.tensor.matmul(
                    rp[:, :1], ones_row[:1], recip[:1, :1], start=True, stop=True
                )
                rep = batchp.tile([P, 1], f32, tag="rep", name="rep")
                nc.vector.tensor_copy(rep[:], rp[:, :1])
                alpha_sb = batchp.tile([P, CW], f32, tag="alpha_sb", name="alpha_sb")
                nc.vector.tensor_scalar_mul(alpha_sb[:], s["w_sb"][:], rep[:, 0:1])
                nc.gpsimd.dma_start(
                    oalpha.ap()[b].rearrange("(t j p) -> p (t j)", j=4, p=P),
                    alpha_sb[:],
                )
                ctx_sb = batchp.tile([1, F], f32, tag="ctx_sb", name="ctx_sb")
                nc.scalar.mul(ctx_sb[:1], s["pc"][:1], recip[:1, 0:1])
                nc.gpsimd.dma_start(octx.ap()[b], ctx_sb[:1])

            # Emission: batch 0's leftovers (lagged exp/ctx + softmax tail)
            # are emitted inside batch 1's first chunk so the PE stream always
            # has runnable matmuls ahead of every dependency chain.
            start_batch(0)
            for lc in range(len(chunk_plans[0])):
                phase1_chunk(0, lc)
            start_batch(1)
            phase1_chunk(1, 0)
            finish_batch(0)
            for lc in range(1, len(chunk_plans[1])):
                phase1_chunk(1, lc)
            finish_batch(1)

    nc.compile()
    _CACHE["nc"] = nc
    return nc


def kernel(z, features, W_feat, W_inp, b_inp, v_atten):
    z = np.asarray(z, dtype=np.float32)
    features = np.asarray(features, dtype=np.float32)
    W_feat = np.asarray(W_feat, dtype=np.float32)
    W_inp = np.asarray(W_inp, dtype=np.float32)
    b_inp = np.asarray(b_inp, dtype=np.float32)
    v_atten = np.asarray(v_atten, dtype=np.float32)

    nc = _build()

    fn_host = features.astype(BF16)  # [B, F, L]
    # ft[b, p, c, f] = features[b, f, 512*(c//4) + 128*(c%4) + p]
    ft_host = np.ascontiguousarray(
        features.reshape(B, F, NT, 4, P).transpose(0, 4, 2, 3, 1).reshape(
            B, P, CW, F
        )
    ).astype(BF16)
    wf_host = np.ascontiguousarray(
        W_feat.T.reshape(NKF, P, A).transpose(1, 0, 2)
    ).astype(BF16)
    wi_host = np.ascontiguousarray(W_inp.T.reshape(NKF, P, A).transpose(1, 0, 2))
    bic_host = np.ascontiguousarray(b_inp.reshape(NA, P).T)
    v_host = np.ascontiguousarray(v_atten.reshape(NA, P).T).astype(BF16)

    in_maps = []
    for i in range(N_CORES):
        sl = slice(i * B_LOC, (i + 1) * B_LOC)
        zt_host = np.ascontiguousarray(
            z[sl].reshape(B_LOC, NKF, P).transpose(2, 1, 0)
        )
        in_maps.append(
            {
                "fn": fn_host[sl],
                "ft": ft_host[sl],
                "wf": wf_host,
                "wi": wi_host,
                "zt": zt_host,
                "bic": bic_host,
                "v": v_host,
            }
        )

    res = run_bass_kernel_spmd(nc, in_maps, list(range(N_CORES)))

    ctx = np.empty((B, F), dtype=np.float32)
    alpha = np.empty((B, L), dtype=np.float32)
    for i in range(N_CORES):
        sl = slice(i * B_LOC, (i + 1) * B_LOC)
        ctx[sl] = res.results[i]["ctx"]
        alpha[sl] = res.results[i]["alpha"]
    return ctx, alpha


# revision 36
# speedup vs baseline: 1.3315x; 1.3315x over previous
"""Trainium2 Bass kernel for attention pooling (sparse_attention).

Computation (per batch b):
    proj_feat = einsum("fl,af->la", features[b], W_feat)        # [L, A]
    p         = z[b] @ W_inp.T + b_inp                          # [A]
    att       = relu(proj_feat + p)                             # [L, A]
    scores    = att @ v_atten                                   # [L]
    alpha     = softmax(scores)                                 # [L]
    ctx       = features[b] @ alpha                             # [F]

Sharding: data-parallel over batch B=16 across 8 cores (2 batches/core).
All parameters replicated. Features are converted to bf16 on the host and
staged in two layouts (natural [F, L] for the projection matmul, and a
transposed/permuted [p, c, F] layout for the context matmul) so that both
big contractions run on the TensorEngine with the contraction dimension on
partitions. Softmax skips max-subtraction (scores are O(+-5), exp is safe
in fp32).
"""

import os
import sys

for _p in ("/root/.axon_site/_ro/trn_rl_repo", "/opt/trn_rl_repo"):
    if os.path.isdir(_p) and _p not in sys.path:
        sys.path.append(_p)

import ml_dtypes
import numpy as np

import concourse.bass as bass  # noqa: F401  (registers engine classes)
import concourse.tile as tile
from concourse import bacc, mybir
from concourse.bass_utils import run_bass_kernel_spmd

BF16 = ml_dtypes.bfloat16

N_CORES = 8
B = 16
B_LOC = B // N_CORES  # 2 batches per core
F = 512
L = 8192
A = 256
I = 512

P = 128
NKF = F // P          # 4 F-chunks (contraction for proj matmul)
NA = A // P           # 2 A-chunks
LC = 2048             # fn DMA chunk along L
NLC = L // LC         # 4
TS = 512              # matmul L-subtile (one PSUM bank)
NT = L // TS          # 16 score tiles per batch
CW = 64               # scores2d free width; l = 64*p + c
NCC = L // P // CW * CW  # = 64 ctx chunks (one per c)

_CACHE = {}


def _build():
    if "nc" in _CACHE:
        return _CACHE["nc"]

    f32 = mybir.dt.float32
    bf16 = mybir.dt.bfloat16
    AF = mybir.ActivationFunctionType

    nc = bacc.Bacc("TRN2", target_bir_lowering=False, debug=False)

    fn = nc.dram_tensor("fn", [B_LOC, F, L], bf16, kind="ExternalInput")
    ft = nc.dram_tensor("ft", [B_LOC, P, CW, F], bf16, kind="ExternalInput")
    wf = nc.dram_tensor("wf", [P, NKF, A], bf16, kind="ExternalInput")
    wi = nc.dram_tensor("wi", [P, NKF, A], f32, kind="ExternalInput")
    zt = nc.dram_tensor("zt", [P, NKF, B_LOC], f32, kind="ExternalInput")
    bic = nc.dram_tensor("bic", [P, NA], f32, kind="ExternalInput")
    vv = nc.dram_tensor("v", [P, NA], bf16, kind="ExternalInput")
    octx = nc.dram_tensor("ctx", [B_LOC, F], f32, kind="ExternalOutput")
    oalpha = nc.dram_tensor("alpha", [B_LOC, L], f32, kind="ExternalOutput")

    with tile.TileContext(nc) as tc:
        with (
            tc.tile_pool(name="consts", bufs=1) as consts,
            tc.tile_pool(name="fnp", bufs=4) as fnp,
            tc.tile_pool(name="ftp", bufs=6) as ftp,
            tc.tile_pool(name="attp", bufs=4) as attp,
            tc.tile_pool(name="smallp", bufs=4) as smallp,
            tc.tile_pool(name="batchp", bufs=2) as batchp,
            tc.tile_pool(name="psS", bufs=5, space="PSUM") as psS,
            tc.tile_pool(name="psV", bufs=1, space="PSUM") as psV,
            tc.tile_pool(name="psC", bufs=1, space="PSUM") as psC,
            tc.tile_pool(name="psT", bufs=1, space="PSUM") as psT,
        ):
            # ---- constants / setup ----
            wf_sb = consts.tile([P, NKF * A], bf16)
            nc.scalar.dma_start(wf_sb[:], wf.ap()[:, :, :])
            wi_sb = consts.tile([P, NKF * A], f32)
            nc.scalar.dma_start(wi_sb[:], wi.ap()[:, :, :])
            zt_sb = consts.tile([P, NKF * B_LOC], f32)
            nc.scalar.dma_start(zt_sb[:], zt.ap()[:, :, :])
            bic_sb = consts.tile([P, NA], f32)
            nc.scalar.dma_start(bic_sb[:], bic.ap()[:, :])
            v_sb = consts.tile([P, NA], bf16)
            nc.scalar.dma_start(v_sb[:], vv.ap()[:, :])
            ones_col = consts.tile([P, 1], f32)
            nc.any.memset(ones_col[:], 1.0)
            ones_row = consts.tile([1, P], f32)
            nc.any.memset(ones_row[:1], 1.0)

            # proj_inp^T: pT[p, a*B_LOC + b] = (z @ W_inp.T + b_inp)[b, a*128+p]
            pT_sb = consts.tile([P, NA * B_LOC], f32)
            for a in range(NA):
                pt_ps = psT.tile([P, B_LOC], f32, tag="tiny")
                for ki in range(NKF):
                    nc.tensor.matmul(
                        pt_ps[:, :B_LOC],
                        wi_sb[:, ki * A + a * P : ki * A + a * P + P],
                        zt_sb[:, ki * B_LOC : (ki + 1) * B_LOC],
                        start=(ki == 0),
                        stop=(ki == NKF - 1),
                    )
                nc.scalar.activation(
                    pT_sb[:, a * B_LOC : (a + 1) * B_LOC],
                    pt_ps[:, :B_LOC],
                    AF.Identity,
                    bias=bic_sb[:, a : a + 1],
                )

            # fn chunk plans (elements along L). Batch 0 ramps in with small
            # chunks to cut time-to-first-matmul; steady state uses 2048.
            # Each chunk is ONE consolidated DMA [128, 4*lsz] (free = (kf, l)).
            # ALL feature traffic (fn and ft) goes on the sync HWDGE ring:
            # ring FIFO is the only real priority control under Tile's
            # dependency scheduler, so the ring order IS the consumption order.
            chunk_plans = {
                0: [1024, 1024, 2048, 2048, 2048],
                1: [2048, 2048, 2048, 2048],
            }
            # ft streams on the scalar HWDGE ring in 2 MB quarters. The
            # modeled-time floors (ms) keep ft quiet while batch 0's fn burst
            # needs the full HBM bandwidth; afterwards the SDMA round-robin
            # splits bandwidth between the fn and ft queues adaptively.
            ft_floor = {0: [0.033, 0.037, 0.041, 0.045],
                        1: [0.052, 0.059, 0.066, 0.072]}
            QF = CW // 4  # c-blocks per ft quarter

            st = {}  # per-batch state

            def start_batch(b):
                s = {}
                s["ft_q"] = []
                for qi in range(4):
                    t = ftp.tile([P, QF * F], bf16, tag="ftq")
                    with tc.tile_wait_until(ft_floor[b][qi]):
                        nc.scalar.dma_start(
                            t[:],
                            ft.ap()[b, :, qi * QF : (qi + 1) * QF, :],
                        )
                    s["ft_q"].append(t)
                s["scores2d"] = batchp.tile(
                    [P, CW], f32, tag="scores2d", name="scores2d"
                )
                s["w_sb"] = batchp.tile([P, CW], f32, tag="w_sb", name="w_sb")
                s["rowsum"] = batchp.tile([P, 1], f32, tag="rowsum", name="rowsum")
                s["w16"] = batchp.tile([P, CW], bf16, tag="w16", name="w16")
                s["fn_src"] = fn.ap()[b].rearrange("(kf p) l -> p kf l", p=P)
                s["l0"] = 0
                st[b] = s

            def phase1_chunk(b, lc):
                s = st[b]
                lsz = chunk_plans[b][lc]
                l0 = s["l0"]
                fn_t = fnp.tile([P, NKF * LC], bf16, tag="fn")
                nc.sync.dma_start(
                    fn_t[:, : NKF * lsz],
                    s["fn_src"][:, :, l0 : l0 + lsz],
                )
                for ts_ in range(lsz // TS):
                    t_idx = l0 // TS + ts_
                    psc = psV.tile([1, TS], f32, tag="psc")
                    for a in range(NA):
                        p1 = psS.tile([P, TS], f32, tag="p1")
                        for kf in range(NKF):
                            nc.tensor.matmul(
                                p1[:],
                                wf_sb[:, kf * A + a * P : kf * A + a * P + P],
                                fn_t[:, kf * lsz + ts_ * TS : kf * lsz + (ts_ + 1) * TS],
                                start=(kf == 0),
                                stop=(kf == NKF - 1),
                            )
                        att_t = attp.tile([P, TS], bf16, tag="att")
                        nc.scalar.activation(
                            att_t[:],
                            p1[:],
                            AF.Relu,
                            bias=pT_sb[:, a * B_LOC + b : a * B_LOC + b + 1],
                        )
                        nc.tensor.matmul(
                            psc[:1],
                            v_sb[:, a : a + 1],
                            att_t[:],
                            start=(a == 0),
                            stop=(a == NA - 1),
                        )
                    sc_row = smallp.tile([1, TS], f32, tag="srow")
                    nc.vector.tensor_copy(sc_row[:1], psc[:1])
                    # scatter [1, 512] -> partitions [8t : 8t+8] x 64
                    nc.gpsimd.dma_start(
                        s["scores2d"][8 * t_idx : 8 * t_idx + 8, :],
                        sc_row[:1],
                    )
                s["l0"] = l0 + lsz

            def softmax(b):
                s = st[b]
                # ---- softmax (no max subtraction needed) ----
                nc.scalar.activation(
                    s["w_sb"][:], s["scores2d"][:], AF.Exp,
                    accum_out=s["rowsum"][:, 0:1],
                )
                nc.vector.tensor_copy(s["w16"][:], s["w_sb"][:])
                zp = psT.tile([P, B_LOC], f32, tag="tiny", name="zp")
                nc.tensor.matmul(
                    zp[:1, :1], ones_col[:], s["rowsum"][:], start=True, stop=True
                )
                recip = batchp.tile([1, 1], f32, tag="recip", name="recip")
                nc.vector.reciprocal(recip[:1, :1], zp[:1, :1])
                rp = psT.tile([P, B_LOC], f32, tag="tiny", name="rp")
                nc.tensor.matmul(
                    rp[:, :1], ones_row[:1], recip[:1, :1], start=True, stop=True
                )
                rep = batchp.tile([P, 1], f32, tag="rep", name="rep")
                nc.vector.tensor_copy(rep[:], rp[:, :1])
                alpha_sb = batchp.tile([P, CW], f32, tag="alpha_sb", name="alpha_sb")
                nc.vector.tensor_scalar_mul(alpha_sb[:], s["w_sb"][:], rep[:, 0:1])
                nc.gpsimd.dma_start(
                    oalpha.ap()[b].rearrange("(p c) -> p c", c=CW), alpha_sb[:]
                )
                s["recip"] = recip
                s["pc"] = psC.tile([1, F], f32, tag="pc", name="pc")

            def ctx_part(b, c_lo, c_hi):
                # ---- ctx = features @ alpha via transposed layout ----
                s = st[b]
                pc = s["pc"]
                for c in range(c_lo, c_hi):
                    nc.tensor.matmul(
                        pc[:1],
                        s["w16"][:, c : c + 1],
                        s["ft_q"][c // QF][:, (c % QF) * F : (c % QF + 1) * F],
                        start=(c == 0),
                        stop=(c == CW - 1),
                    )
                if c_hi == CW:
                    ctx_sb = batchp.tile([1, F], f32, tag="ctx_sb", name="ctx_sb")
                    nc.scalar.mul(ctx_sb[:1], pc[:1], s["recip"][:1, 0:1])
                    nc.gpsimd.dma_start(octx.ap()[b], ctx_sb[:1])

            # Emission order interleaves the two batches so the PE stream
            # always has runnable matmuls queued ahead of each softmax
            # dependency chain (PE is strict FIFO: a waiting instruction
            # blocks everything behind it).
            start_batch(0)
            for lc in range(len(chunk_plans[0])):
                phase1_chunk(0, lc)
            start_batch(1)
            phase1_chunk(1, 0)
            phase1_chunk(1, 1)
            phase1_chunk(1, 2)
            softmax(0)
            ctx_part(0, 0, CW // 2)
            phase1_chunk(1, 3)
            ctx_part(0, CW // 2, CW)
            softmax(1)
            ctx_part(1, 0, CW)

    nc.compile()
    _CACHE["nc"] = nc
    return nc


def kernel(z, features, W_feat, W_inp, b_inp, v_atten):
    z = np.asarray(z, dtype=np.float32)
    features = np.asarray(features, dtype=np.float32)
    W_feat = np.asarray(W_feat, dtype=np.float32)
    W_inp = np.asarray(W_inp, dtype=np.float32)
    b_inp = np.asarray(b_inp, dtype=np.float32)
    v_atten = np.asarray(v_atten, dtype=np.float32)

    nc = _build()

    fn_host = features.astype(BF16)  # [B, F, L]
    # ft[b, p, c, f] = features[b, f, 64*p + c]
    ft_host = np.ascontiguousarray(
        features.reshape(B, F, P, CW).transpose(0, 2, 3, 1)
    ).astype(BF16)
    wf_host = np.ascontiguousarray(
        W_feat.T.reshape(NKF, P, A).transpose(1, 0, 2)
    ).astype(BF16)
    wi_host = np.ascontiguousarray(W_inp.T.reshape(NKF, P, A).transpose(1, 0, 2))
    bic_host = np.ascontiguousarray(b_inp.reshape(NA, P).T)
    v_host = np.ascontiguousarray(v_atten.reshape(NA, P).T).astype(BF16)

    in_maps = []
    for i in range(N_CORES):
        sl = slice(i * B_LOC, (i + 1) * B_LOC)
        zt_host = np.ascontiguousarray(
            z[sl].reshape(B_LOC, NKF, P).transpose(2, 1, 0)
        )
        in_maps.append(
            {
                "fn": fn_host[sl],
                "ft": ft_host[sl],
                "wf": wf_host,
                "wi": wi_host,
                "zt": zt_host,
                "bic": bic_host,
                "v": v_host,
            }
        )

    res = run_bass_kernel_spmd(nc, in_maps, list(range(N_CORES)))

    ctx = np.empty((B, F), dtype=np.float32)
    alpha = np.empty((B, L), dtype=np.float32)
    for i in range(N_CORES):
        sl = slice(i * B_LOC, (i + 1) * B_LOC)
        ctx[sl] = res.results[i]["ctx"]
        alpha[sl] = res.results[i]["alpha"]
    return ctx, alpha


# revision 38
# speedup vs baseline: 1.3750x; 1.0327x over previous
"""Trainium2 Bass kernel for attention pooling (sparse_attention).

Computation (per batch b):
    proj_feat = einsum("fl,af->la", features[b], W_feat)        # [L, A]
    p         = z[b] @ W_inp.T + b_inp                          # [A]
    att       = relu(proj_feat + p)                             # [L, A]
    scores    = att @ v_atten                                   # [L]
    alpha     = softmax(scores)                                 # [L]
    ctx       = features[b] @ alpha                             # [F]

Sharding: data-parallel over batch B=16 across 8 cores (2 batches/core).
All parameters replicated. Features are converted to bf16 on the host and
staged in two layouts (natural [F, L] for the projection matmul, and a
transposed/permuted [p, c, F] layout for the context matmul) so that both
big contractions run on the TensorEngine with the contraction dimension on
partitions. Softmax skips max-subtraction (scores are O(+-5), exp is safe
in fp32).
"""

import os
import sys

for _p in ("/root/.axon_site/_ro/trn_rl_repo", "/opt/trn_rl_repo"):
    if os.path.isdir(_p) and _p not in sys.path:
        sys.path.append(_p)

import ml_dtypes
import numpy as np

import concourse.bass as bass  # noqa: F401  (registers engine classes)
import concourse.tile as tile
from concourse import bacc, mybir
from concourse.bass_utils import run_bass_kernel_spmd
from concourse.tile import add_dep_helper

BF16 = ml_dtypes.bfloat16

N_CORES = 8
B = 16
B_LOC = B // N_CORES  # 2 batches per core
F = 512
L = 8192
A = 256
I = 512

P = 128
NKF = F // P          # 4 F-chunks (contraction for proj matmul)
NA = A // P           # 2 A-chunks
LC = 2048             # fn DMA chunk along L
NLC = L // LC         # 4
TS = 512              # matmul L-subtile (one PSUM bank)
NT = L // TS          # 16 score tiles per batch
CW = 64               # scores2d free width; l = 64*p + c
NCC = L // P // CW * CW  # = 64 ctx chunks (one per c)

_CACHE = {}


def _build():
    if "nc" in _CACHE:
        return _CACHE["nc"]

    f32 = mybir.dt.float32
    bf16 = mybir.dt.bfloat16
    AF = mybir.ActivationFunctionType

    nc = bacc.Bacc("TRN2", target_bir_lowering=False, debug=False)

    fn = nc.dram_tensor("fn", [B_LOC, F, L], bf16, kind="ExternalInput")
    ft = nc.dram_tensor("ft", [B_LOC, P, CW, F], bf16, kind="ExternalInput")
    wf = nc.dram_tensor("wf", [P, NKF, A], bf16, kind="ExternalInput")
    wi = nc.dram_tensor("wi", [P, NKF, A], f32, kind="ExternalInput")
    zt = nc.dram_tensor("zt", [P, NKF, B_LOC], f32, kind="ExternalInput")
    bic = nc.dram_tensor("bic", [P, NA], f32, kind="ExternalInput")
    vv = nc.dram_tensor("v", [P, NA], bf16, kind="ExternalInput")
    octx = nc.dram_tensor("ctx", [B_LOC, F], f32, kind="ExternalOutput")
    oalpha = nc.dram_tensor("alpha", [B_LOC, L], f32, kind="ExternalOutput")

    with tile.TileContext(nc) as tc:
        with (
            tc.tile_pool(name="consts", bufs=1) as consts,
            tc.tile_pool(name="fnp", bufs=4) as fnp,
            tc.tile_pool(name="ftp", bufs=6) as ftp,
            tc.tile_pool(name="attp", bufs=4) as attp,
            tc.tile_pool(name="smallp", bufs=4) as smallp,
            tc.tile_pool(name="batchp", bufs=2) as batchp,
            tc.tile_pool(name="psS", bufs=5, space="PSUM") as psS,
            tc.tile_pool(name="psV", bufs=1, space="PSUM") as psV,
            tc.tile_pool(name="psC", bufs=1, space="PSUM") as psC,
            tc.tile_pool(name="psT", bufs=1, space="PSUM") as psT,
        ):
            # ---- constants / setup ----
            # wi/zt/bic first: they gate the pT matmuls which sit at the head
            # of the PE stream; wf only gates the first S-matmul (later).
            wi_sb = consts.tile([P, NKF * A], f32)
            nc.scalar.dma_start(wi_sb[:], wi.ap()[:, :, :])
            zt_sb = consts.tile([P, NKF * B_LOC], f32)
            nc.scalar.dma_start(zt_sb[:], zt.ap()[:, :, :])
            bic_sb = consts.tile([P, NA], f32)
            nc.scalar.dma_start(bic_sb[:], bic.ap()[:, :])
            wf_sb = consts.tile([P, NKF * A], bf16)
            nc.scalar.dma_start(wf_sb[:], wf.ap()[:, :, :])
            v_sb = consts.tile([P, NA], bf16)
            nc.scalar.dma_start(v_sb[:], vv.ap()[:, :])
            ones_col = consts.tile([P, 1], f32)
            nc.any.memset(ones_col[:], 1.0)
            ones_row = consts.tile([1, P], f32)
            nc.any.memset(ones_row[:1], 1.0)

            # proj_inp^T: pT[p, a*B_LOC + b] = (z @ W_inp.T + b_inp)[b, a*128+p]
            pT_sb = consts.tile([P, NA * B_LOC], f32)

            def emit_pT():
                for a in range(NA):
                    pt_ps = psT.tile([P, B_LOC], f32, tag="tiny", name="pt_ps")
                    for ki in range(NKF):
                        nc.tensor.matmul(
                            pt_ps[:, :B_LOC],
                            wi_sb[:, ki * A + a * P : ki * A + a * P + P],
                            zt_sb[:, ki * B_LOC : (ki + 1) * B_LOC],
                            start=(ki == 0),
                            stop=(ki == NKF - 1),
                        )
                    nc.scalar.activation(
                        pT_sb[:, a * B_LOC : (a + 1) * B_LOC],
                        pt_ps[:, :B_LOC],
                        AF.Identity,
                        bias=bic_sb[:, a : a + 1],
                    )

            # fn chunk plans (elements along L). Batch 0 ramps in with small
            # chunks to cut time-to-first-matmul; steady state uses 2048.
            # Each chunk is ONE consolidated DMA [128, 4*lsz] (free = (kf, l)).
            # ALL feature traffic (fn and ft) goes on the sync HWDGE ring:
            # ring FIFO is the only real priority control under Tile's
            # dependency scheduler, so the ring order IS the consumption order.
            chunk_plans = {
                0: [512, 1536, 2048, 2048, 2048],
                1: [2048, 2048, 2048, 2048],
            }
            # ft quarters issue after these fn chunks (real DMA deps)
            ft_plan = {0: {2: [0], 3: [1], 4: [2, 3]},
                       1: {0: [0], 1: [1], 2: [2], 3: [3]}}
            QF = CW // 4  # c-blocks per ft quarter

            st = {}  # per-batch state

            def start_batch(b):
                s = {}
                s["ft_q"] = [
                    ftp.tile([P, QF * F], bf16, tag="ftq", name="ftq")
                    for _ in range(4)
                ]
                s["scores2d"] = batchp.tile(
                    [P, CW], f32, tag="scores2d", name="scores2d"
                )
                s["w_sb"] = batchp.tile([P, CW], f32, tag="w_sb", name="w_sb")
                s["rowsum"] = batchp.tile([P, 1], f32, tag="rowsum", name="rowsum")
                s["w16"] = batchp.tile([P, CW], bf16, tag="w16", name="w16")
                s["fn_src"] = fn.ap()[b].rearrange("(kf p) l -> p kf l", p=P)
                s["l0"] = 0
                st[b] = s

            def phase1_chunk(b, lc):
                s = st[b]
                lsz = chunk_plans[b][lc]
                l0 = s["l0"]
                fn_t = fnp.tile([P, NKF * LC], bf16, tag="fn")
                fn_dma = nc.sync.dma_start(
                    fn_t[:, : NKF * lsz],
                    s["fn_src"][:, :, l0 : l0 + lsz],
                )
                for qi in ft_plan[b].get(lc, []):
                    ft_dma = nc.gpsimd.dma_start(
                        s["ft_q"][qi][:],
                        ft.ap()[b, :, qi * QF : (qi + 1) * QF, :],
                    )
                    add_dep_helper(
                        ft_dma.ins, fn_dma.ins, sync=True,
                        reason="pace ft behind the fn stream",
                    )
                for ts_ in range(lsz // TS):
                    t_idx = l0 // TS + ts_
                    psc = psV.tile([1, TS], f32, tag="psc")
                    for a in range(NA):
                        p1 = psS.tile([P, TS], f32, tag="p1")
                        for kf in range(NKF):
                            nc.tensor.matmul(
                                p1[:],
                                wf_sb[:, kf * A + a * P : kf * A + a * P + P],
                                fn_t[:, kf * lsz + ts_ * TS : kf * lsz + (ts_ + 1) * TS],
                                start=(kf == 0),
                                stop=(kf == NKF - 1),
                            )
                        att_t = attp.tile([P, TS], bf16, tag="att")
                        nc.scalar.activation(
                            att_t[:],
                            p1[:],
                            AF.Relu,
                            bias=pT_sb[:, a * B_LOC + b : a * B_LOC + b + 1],
                        )
                        nc.tensor.matmul(
                            psc[:1],
                            v_sb[:, a : a + 1],
                            att_t[:],
                            start=(a == 0),
                            stop=(a == NA - 1),
                        )
                    sc_row = smallp.tile([1, TS], f32, tag="srow")
                    nc.vector.tensor_copy(sc_row[:1], psc[:1])
                    # scatter [1, 512] -> partitions [8t : 8t+8] x 64
                    nc.gpsimd.dma_start(
                        s["scores2d"][8 * t_idx : 8 * t_idx + 8, :],
                        sc_row[:1],
                    )
                s["l0"] = l0 + lsz

            def softmax(b):
                s = st[b]
                # ---- softmax (no max subtraction needed) ----
                nc.scalar.activation(
                    s["w_sb"][:], s["scores2d"][:], AF.Exp,
                    accum_out=s["rowsum"][:, 0:1],
                )
                nc.vector.tensor_copy(s["w16"][:], s["w_sb"][:])
                zp = psT.tile([P, B_LOC], f32, tag="tiny", name="zp")
                nc.tensor.matmul(
                    zp[:1, :1], ones_col[:], s["rowsum"][:], start=True, stop=True
                )
                recip = batchp.tile([1, 1], f32, tag="recip", name="recip")
                nc.vector.reciprocal(recip[:1, :1], zp[:1, :1])
                rp = psT.tile([P, B_LOC], f32, tag="tiny", name="rp")
                nc.tensor.matmul(
                    rp[:, :1], ones_row[:1], recip[:1, :1], start=True, stop=True
                )
                rep = batchp.tile([P, 1], f32, tag="rep", name="rep")
                nc.vector.tensor_copy(rep[:], rp[:, :1])
                alpha_sb = batchp.tile([P, CW], f32, tag="alpha_sb", name="alpha_sb")
                nc.vector.tensor_scalar_mul(alpha_sb[:], s["w_sb"][:], rep[:, 0:1])
                nc.gpsimd.dma_start(
                    oalpha.ap()[b].rearrange("(p c) -> p c", c=CW), alpha_sb[:]
                )
                s["recip"] = recip
                s["pc"] = psC.tile([1, F], f32, tag="pc", name="pc")

            def ctx_part(b, c_lo, c_hi):
                # ---- ctx = features @ alpha via transposed layout ----
                s = st[b]
                pc = s["pc"]
                for c in range(c_lo, c_hi):
                    nc.tensor.matmul(
                        pc[:1],
                        s["w16"][:, c : c + 1],
                        s["ft_q"][c // QF][:, (c % QF) * F : (c % QF + 1) * F],
                        start=(c == 0),
                        stop=(c == CW - 1),
                    )
                if c_hi == CW:
                    ctx_sb = batchp.tile([1, F], f32, tag="ctx_sb", name="ctx_sb")
                    nc.scalar.mul(ctx_sb[:1], pc[:1], s["recip"][:1, 0:1])
                    nc.gpsimd.dma_start(octx.ap()[b], ctx_sb[:1])

            # Emission order interleaves the two batches so the PE stream
            # always has runnable matmuls queued ahead of each softmax
            # dependency chain (PE is strict FIFO: a waiting instruction
            # blocks everything behind it).
            emit_pT()
            start_batch(0)
            for lc in range(len(chunk_plans[0])):
                phase1_chunk(0, lc)
            start_batch(1)
            phase1_chunk(1, 0)
            phase1_chunk(1, 1)
            phase1_chunk(1, 2)
            softmax(0)
            ctx_part(0, 0, CW // 2)
            phase1_chunk(1, 3)
            ctx_part(0, CW // 2, CW)
            softmax(1)
            ctx_part(1, 0, CW)

    nc.compile()
    _CACHE["nc"] = nc
    return nc


def kernel(z, features, W_feat, W_inp, b_inp, v_atten):
    z = np.asarray(z, dtype=np.float32)
    features = np.asarray(features, dtype=np.float32)
    W_feat = np.asarray(W_feat, dtype=np.float32)
    W_inp = np.asarray(W_inp, dtype=np.float32)
    b_inp = np.asarray(b_inp, dtype=np.float32)
    v_atten = np.asarray(v_atten, dtype=np.float32)

    nc = _build()

    fn_host = features.astype(BF16)  # [B, F, L]
    # ft[b, p, c, f] = features[b, f, 64*p + c]
    ft_host = np.ascontiguousarray(
        features.reshape(B, F, P, CW).transpose(0, 2, 3, 1)
    ).astype(BF16)
    wf_host = np.ascontiguousarray(
        W_feat.T.reshape(NKF, P, A).transpose(1, 0, 2)
    ).astype(BF16)
    wi_host = np.ascontiguousarray(W_inp.T.reshape(NKF, P, A).transpose(1, 0, 2))
    bic_host = np.ascontiguousarray(b_inp.reshape(NA, P).T)
    v_host = np.ascontiguousarray(v_atten.reshape(NA, P).T).astype(BF16)

    in_maps = []
    for i in range(N_CORES):
        sl = slice(i * B_LOC, (i + 1) * B_LOC)
        zt_host = np.ascontiguousarray(
            z[sl].reshape(B_LOC, NKF, P).transpose(2, 1, 0)
        )
        in_maps.append(
            {
                "fn": fn_host[sl],
                "ft": ft_host[sl],
                "wf": wf_host,
                "wi": wi_host,
                "zt": zt_host,
                "bic": bic_host,
                "v": v_host,
            }
        )

    res = run_bass_kernel_spmd(nc, in_maps, list(range(N_CORES)))

    ctx = np.empty((B, F), dtype=np.float32)
    alpha = np.empty((B, L), dtype=np.float32)
    for i in range(N_CORES):
        sl = slice(i * B_LOC, (i + 1) * B_LOC)
        ctx[sl] = res.results[i]["ctx"]
        alpha[sl] = res.results[i]["alpha"]
    return ctx, alpha


# revision 39
# speedup vs baseline: 1.4718x; 1.0704x over previous
"""Trainium2 Bass kernel for attention pooling (sparse_attention).

Computation (per batch b):
    proj_feat = einsum("fl,af->la", features[b], W_feat)        # [L, A]
    p         = z[b] @ W_inp.T + b_inp                          # [A]
    att       = relu(proj_feat + p)                             # [L, A]
    scores    = att @ v_atten                                   # [L]
    alpha     = softmax(scores)                                 # [L]
    ctx       = features[b] @ alpha                             # [F]

Sharding: data-parallel over batch B=16 across 8 cores (2 batches/core).
All parameters replicated. Features are converted to bf16 on the host and
staged in two layouts (natural [F, L] for the projection matmul, and a
transposed/permuted [p, c, F] layout for the context matmul) so that both
big contractions run on the TensorEngine with the contraction dimension on
partitions. Softmax skips max-subtraction (scores are O(+-5), exp is safe
in fp32).
"""

import os
import sys

for _p in ("/root/.axon_site/_ro/trn_rl_repo", "/opt/trn_rl_repo"):
    if os.path.isdir(_p) and _p not in sys.path:
        sys.path.append(_p)

import ml_dtypes
import numpy as np

import concourse.bass as bass  # noqa: F401  (registers engine classes)
import concourse.tile as tile
from concourse import bacc, mybir
from concourse.bass_utils import run_bass_kernel_spmd
from concourse.tile import add_dep_helper

BF16 = ml_dtypes.bfloat16

N_CORES = 8
B = 16
B_LOC = B // N_CORES  # 2 batches per core
F = 512
L = 8192
A = 256
I = 512

P = 128
NKF = F // P          # 4 F-chunks (contraction for proj matmul)
NA = A // P           # 2 A-chunks
LC = 2048             # fn DMA chunk along L
NLC = L // LC         # 4
TS = 512              # matmul L-subtile (one PSUM bank)
NT = L // TS          # 16 score tiles per batch
CW = 64               # scores2d free width; l = 64*p + c
NCC = L // P // CW * CW  # = 64 ctx chunks (one per c)

_CACHE = {}


def _build():
    if "nc" in _CACHE:
        return _CACHE["nc"]

    f32 = mybir.dt.float32
    bf16 = mybir.dt.bfloat16
    AF = mybir.ActivationFunctionType

    nc = bacc.Bacc("TRN2", target_bir_lowering=False, debug=False)

    fn = nc.dram_tensor("fn", [B_LOC, F, L], bf16, kind="ExternalInput")
    ft = nc.dram_tensor("ft", [B_LOC, P, CW, F], bf16, kind="ExternalInput")
    wf = nc.dram_tensor("wf", [P, NKF, A], bf16, kind="ExternalInput")
    wi = nc.dram_tensor("wi", [P, NKF, A], f32, kind="ExternalInput")
    zt = nc.dram_tensor("zt", [P, NKF, B_LOC], f32, kind="ExternalInput")
    bic = nc.dram_tensor("bic", [P, NA], f32, kind="ExternalInput")
    vv = nc.dram_tensor("v", [P, NA], bf16, kind="ExternalInput")
    octx = nc.dram_tensor("ctx", [B_LOC, F], f32, kind="ExternalOutput")
    oalpha = nc.dram_tensor("alpha", [B_LOC, L], f32, kind="ExternalOutput")

    with tile.TileContext(nc) as tc:
        with (
            tc.tile_pool(name="consts", bufs=1) as consts,
            tc.tile_pool(name="fnp", bufs=4) as fnp,
            tc.tile_pool(name="ftp", bufs=6) as ftp,
            tc.tile_pool(name="attp", bufs=4) as attp,
            tc.tile_pool(name="smallp", bufs=4) as smallp,
            tc.tile_pool(name="batchp", bufs=2) as batchp,
            tc.tile_pool(name="psS", bufs=5, space="PSUM") as psS,
            tc.tile_pool(name="psV", bufs=1, space="PSUM") as psV,
            tc.tile_pool(name="psC", bufs=1, space="PSUM") as psC,
            tc.tile_pool(name="psT", bufs=1, space="PSUM") as psT,
        ):
            # ---- constants / setup ----
            # wi/zt/bic first: they gate the pT matmuls which sit at the head
            # of the PE stream; wf only gates the first S-matmul (later).
            wi_sb = consts.tile([P, NKF * A], f32)
            nc.sync.dma_start(wi_sb[:], wi.ap()[:, :, :])
            zt_sb = consts.tile([P, NKF * B_LOC], f32)
            nc.sync.dma_start(zt_sb[:], zt.ap()[:, :, :])
            bic_sb = consts.tile([P, NA], f32)
            nc.sync.dma_start(bic_sb[:], bic.ap()[:, :])
            wf_sb = consts.tile([P, NKF * A], bf16)
            nc.scalar.dma_start(wf_sb[:], wf.ap()[:, :, :])
            v_sb = consts.tile([P, NA], bf16)
            nc.scalar.dma_start(v_sb[:], vv.ap()[:, :])
            ones_col = consts.tile([P, 1], f32)
            nc.any.memset(ones_col[:], 1.0)
            ones_row = consts.tile([1, P], f32)
            nc.any.memset(ones_row[:1], 1.0)

            # proj_inp^T: pT[p, a*B_LOC + b] = (z @ W_inp.T + b_inp)[b, a*128+p]
            pT_sb = consts.tile([P, NA * B_LOC], f32)

            def emit_pT():
                for a in range(NA):
                    pt_ps = psT.tile([P, B_LOC], f32, tag="tiny", name="pt_ps")
                    for ki in range(NKF):
                        nc.tensor.matmul(
                            pt_ps[:, :B_LOC],
                            wi_sb[:, ki * A + a * P : ki * A + a * P + P],
                            zt_sb[:, ki * B_LOC : (ki + 1) * B_LOC],
                            start=(ki == 0),
                            stop=(ki == NKF - 1),
                        )
                    nc.scalar.activation(
                        pT_sb[:, a * B_LOC : (a + 1) * B_LOC],
                        pt_ps[:, :B_LOC],
                        AF.Identity,
                        bias=bic_sb[:, a : a + 1],
                    )

            # fn chunk plans (elements along L). Batch 0 ramps in with small
            # chunks to cut time-to-first-matmul; steady state uses 2048.
            # Each chunk is ONE consolidated DMA [128, 4*lsz] (free = (kf, l)).
            # ALL feature traffic (fn and ft) goes on the sync HWDGE ring:
            # ring FIFO is the only real priority control under Tile's
            # dependency scheduler, so the ring order IS the consumption order.
            chunk_plans = {
                0: [512, 1536, 2048, 2048, 2048],
                1: [2048, 2048, 2048, 2048],
            }
            # ft quarters issue after these fn chunks (real DMA deps)
            ft_plan = {0: {2: [0], 3: [1], 4: [2, 3]},
                       1: {0: [0], 1: [1], 2: [2], 3: [3]}}
            QF = CW // 4  # c-blocks per ft quarter

            st = {}  # per-batch state

            def start_batch(b):
                s = {}
                s["ft_q"] = [
                    ftp.tile([P, QF * F], bf16, tag="ftq", name="ftq")
                    for _ in range(4)
                ]
                s["scores2d"] = batchp.tile(
                    [P, CW], f32, tag="scores2d", name="scores2d"
                )
                s["w_sb"] = batchp.tile([P, CW], f32, tag="w_sb", name="w_sb")
                s["rowsum"] = batchp.tile([P, 1], f32, tag="rowsum", name="rowsum")
                s["w16"] = batchp.tile([P, CW], bf16, tag="w16", name="w16")
                s["fn_src"] = fn.ap()[b].rearrange("(kf p) l -> p kf l", p=P)
                s["l0"] = 0
                st[b] = s

            def phase1_chunk(b, lc):
                s = st[b]
                lsz = chunk_plans[b][lc]
                l0 = s["l0"]
                fn_t = fnp.tile([P, NKF * LC], bf16, tag="fn")
                fn_dma = nc.sync.dma_start(
                    fn_t[:, : NKF * lsz],
                    s["fn_src"][:, :, l0 : l0 + lsz],
                )
                for qi in ft_plan[b].get(lc, []):
                    ft_dma = nc.gpsimd.dma_start(
                        s["ft_q"][qi][:],
                        ft.ap()[b, :, qi * QF : (qi + 1) * QF, :],
                    )
                    add_dep_helper(
                        ft_dma.ins, fn_dma.ins, sync=True,
                        reason="pace ft behind the fn stream",
                    )
                for ts_ in range(lsz // TS):
                    t_idx = l0 // TS + ts_
                    psc = psV.tile([1, TS], f32, tag="psc")
                    for a in range(NA):
                        p1 = psS.tile([P, TS], f32, tag="p1")
                        for kf in range(NKF):
                            nc.tensor.matmul(
                                p1[:],
                                wf_sb[:, kf * A + a * P : kf * A + a * P + P],
                                fn_t[:, kf * lsz + ts_ * TS : kf * lsz + (ts_ + 1) * TS],
                                start=(kf == 0),
                                stop=(kf == NKF - 1),
                            )
                        att_t = attp.tile([P, TS], bf16, tag="att")
                        nc.scalar.activation(
                            att_t[:],
                            p1[:],
                            AF.Relu,
                            bias=pT_sb[:, a * B_LOC + b : a * B_LOC + b + 1],
                        )
                        nc.tensor.matmul(
                            psc[:1],
                            v_sb[:, a : a + 1],
                            att_t[:],
                            start=(a == 0),
                            stop=(a == NA - 1),
                        )
                    sc_row = smallp.tile([1, TS], f32, tag="srow")
                    nc.vector.tensor_copy(sc_row[:1], psc[:1])
                    # scatter [1, 512] -> partitions [8t : 8t+8] x 64
                    nc.gpsimd.dma_start(
                        s["scores2d"][8 * t_idx : 8 * t_idx + 8, :],
                        sc_row[:1],
                    )
                s["l0"] = l0 + lsz

            def softmax(b):
                s = st[b]
                # ---- softmax (no max subtraction needed) ----
                nc.scalar.activation(
                    s["w_sb"][:], s["scores2d"][:], AF.Exp,
                    accum_out=s["rowsum"][:, 0:1],
                )
                nc.vector.tensor_copy(s["w16"][:], s["w_sb"][:])
                zp = psT.tile([P, B_LOC], f32, tag="tiny", name="zp")
                nc.tensor.matmul(
                    zp[:1, :1], ones_col[:], s["rowsum"][:], start=True, stop=True
                )
                recip = batchp.tile([1, 1], f32, tag="recip", name="recip")
                nc.vector.reciprocal(recip[:1, :1], zp[:1, :1])
                rp = psT.tile([P, B_LOC], f32, tag="tiny", name="rp")
                nc.tensor.matmul(
                    rp[:, :1], ones_row[:1], recip[:1, :1], start=True, stop=True
                )
                rep = batchp.tile([P, 1], f32, tag="rep", name="rep")
                nc.vector.tensor_copy(rep[:], rp[:, :1])
                alpha_sb = batchp.tile([P, CW], f32, tag="alpha_sb", name="alpha_sb")
                nc.vector.tensor_scalar_mul(alpha_sb[:], s["w_sb"][:], rep[:, 0:1])
                nc.gpsimd.dma_start(
                    oalpha.ap()[b].rearrange("(p c) -> p c", c=CW), alpha_sb[:]
                )
                s["recip"] = recip
                s["pc"] = psC.tile([1, F], f32, tag="pc", name="pc")

            def ctx_part(b, c_lo, c_hi):
                # ---- ctx = features @ alpha via transposed layout ----
                s = st[b]
                pc = s["pc"]
                for c in range(c_lo, c_hi):
                    nc.tensor.matmul(
                        pc[:1],
                        s["w16"][:, c : c + 1],
                        s["ft_q"][c // QF][:, (c % QF) * F : (c % QF + 1) * F],
                        start=(c == 0),
                        stop=(c == CW - 1),
                    )
                if c_hi == CW:
                    ctx_sb = batchp.tile([1, F], f32, tag="ctx_sb", name="ctx_sb")
                    nc.scalar.mul(ctx_sb[:1], pc[:1], s["recip"][:1, 0:1])
                    nc.gpsimd.dma_start(octx.ap()[b], ctx_sb[:1])

            # Emission order interleaves the two batches so the PE stream
            # always has runnable matmuls queued ahead of each softmax
            # dependency chain (PE is strict FIFO: a waiting instruction
            # blocks everything behind it).
            emit_pT()
            start_batch(0)
            for lc in range(len(chunk_plans[0])):
                phase1_chunk(0, lc)
            start_batch(1)
            phase1_chunk(1, 0)
            phase1_chunk(1, 1)
            phase1_chunk(1, 2)
            softmax(0)
            ctx_part(0, 0, CW // 2)
            phase1_chunk(1, 3)
            ctx_part(0, CW // 2, CW)
            softmax(1)
            ctx_part(1, 0, CW)

    nc.compile()
    _CACHE["nc"] = nc
    return nc


def kernel(z, features, W_feat, W_inp, b_inp, v_atten):
    z = np.asarray(z, dtype=np.float32)
    features = np.asarray(features, dtype=np.float32)
    W_feat = np.asarray(W_feat, dtype=np.float32)
    W_inp = np.asarray(W_inp, dtype=np.float32)
    b_inp = np.asarray(b_inp, dtype=np.float32)
    v_atten = np.asarray(v_atten, dtype=np.float32)

    nc = _build()

    fn_host = features.astype(BF16)  # [B, F, L]
    # ft[b, p, c, f] = features[b, f, 64*p + c]
    ft_host = np.ascontiguousarray(
        features.reshape(B, F, P, CW).transpose(0, 2, 3, 1)
    ).astype(BF16)
    wf_host = np.ascontiguousarray(
        W_feat.T.reshape(NKF, P, A).transpose(1, 0, 2)
    ).astype(BF16)
    wi_host = np.ascontiguousarray(W_inp.T.reshape(NKF, P, A).transpose(1, 0, 2))
    bic_host = np.ascontiguousarray(b_inp.reshape(NA, P).T)
    v_host = np.ascontiguousarray(v_atten.reshape(NA, P).T).astype(BF16)

    in_maps = []
    for i in range(N_CORES):
        sl = slice(i * B_LOC, (i + 1) * B_LOC)
        zt_host = np.ascontiguousarray(
            z[sl].reshape(B_LOC, NKF, P).transpose(2, 1, 0)
        )
        in_maps.append(
            {
                "fn": fn_host[sl],
                "ft": ft_host[sl],
                "wf": wf_host,
                "wi": wi_host,
                "zt": zt_host,
                "bic": bic_host,
                "v": v_host,
            }
        )

    res = run_bass_kernel_spmd(nc, in_maps, list(range(N_CORES)))

    ctx = np.empty((B, F), dtype=np.float32)
    alpha = np.empty((B, L), dtype=np.float32)
    for i in range(N_CORES):
        sl = slice(i * B_LOC, (i + 1) * B_LOC)
        ctx[sl] = res.results[i]["ctx"]
        alpha[sl] = res.results[i]["alpha"]
    return ctx, alpha
